# revision 34
# baseline (speedup 1.0000x reference)
"""Trainium2 Bass kernel for nn_CascadingSystem (confidence-gated 2-expert blend).

Computation (reference):
    xf = x.reshape(256, 150528)
    t_out = xf @ W1 + b1            # [256, 2]
    f_out = xf @ W2 + b2            # [256, 2]
    conf  = max(softmax(t_out, 1), 1)
    out   = where(conf > 0.95, t_out, 0.7*t_out + 0.3*f_out)

Strategy (memory-bound; reading x dominates; ~410 B/ns effective DMA/core):
  - Shard the feature dim D=150528 across 8 cores (18816 each). Every core
    streams its d-slice of ALL 256 samples once from HBM and computes the
    partial [4, 256] logits (4 = W1c0, W1c1, W2c0, W2c1) on the tensor
    engine, 147 accumulating matmul chunks of K=128.
  - Precision/bandwidth: fp32 matmuls are 4 cyc/row (PE-bound) and fp32
    data is 4 B/elem. Instead decompose on the host
        x = xh(fp16) + xr,   xr8 = fp8_e4m3(xr * 2^12)
        W = wh(fp16) + wl(fp16),  w8 = fp8_e4m3(W * 2^9)
        logits = xh*wh + xh*wl + (xr8*w8) / 2^21
    3 B/elem -> ~36us stream; PE does one fp16 matmul (1 cyc/row) per
    chunk plus one fp8 DoubleRow matmul (0.5 cyc/row, contracts two
    chunks) per chunk pair => ~24us, comfortably chasing the DMA
    roofline. Max logit error ~1.2e-4. DoubleRow needs the fp8 weight
    k-tile step to be a multiple of 16 B (s3_lw_dual_fp8_restrictions),
    so w8 is host-padded to 16 B per chunk.
  - DMA structure (all measured on HW): the two HWDGE queues (sync +
    scalar) share a descriptor-dispatch arbiter; sustained ~415 B/ns needs
    BOTH queues active with many comparable, mid-size entries finely
    interleaved. Coarse multi-MB entries make one queue hog dispatch, and
    small-descriptor entries pay a ~1.3us dispatch floor — see SIZES. Per-
    stream chunk DMAs alternate engines (parity); W goes first as its own
    DMAs; sem clears precede the NRT pseudo-barrier, and all DMA issues
    happen right after it so transfers overlap the PE's startup.
  - Epilogue: vector (DVE — needs no ACT_TABLE_LOAD, unlike the scalar/
    Activation engine) waits on the PE sem, copies both PSUM accumulators
    to out_sb, and its dve_sem handoff doubles as the ordering fence for
    scalar's out DMA — engines run with relaxed ordering, and a same-
    engine dma_start executes AHEAD of in-flight copies and ships stale
    out_sb (observed on HW). Two dummy DVE casts cover the ~50ns window
    between matmul retire (sem update) and the systolic drain landing in
    PSUM. Nobody waits on the out DMA's sem: the Block-exit DRAIN on
    scalar quiesces its HWDGE queue, saving the ~0.9us sem propagation.
  - Host sums the 8 partial tensors and applies the tiny
    bias/softmax/threshold/blend epilogue on [256, 4] floats.

Measured frontier (10+ samples, 52.7-60.9us; machine drifts ~4us in
multi-minute throttle phases): stream 8.5->47.4us (within ~2us of its
floor at the 420-430 B/ns plateau), PE tail +1.8us, epilogue+barriers
3.2us, framework preamble 6.7us fixed. Untested micro-ideas, each
~0.2-0.4us (below the +/-3us run noise) with PSUM-drain tail risk:
(a) split pe_sem into pe16 (after last fp16 matmul) / pe8 (after last
fp8) so the DVE CASTs+acc16 copy overlap the final DoubleRow matmul —
keep one CAST before EACH copy as drain margin; (b) acc8 copy on gpsimd
in parallel with acc16 on DVE (Pool-engine PSUM read unverified).
"""

from contextlib import ExitStack

import ml_dtypes
import numpy as np

import concourse.bass as bass
import concourse.mybir as mybir
from concourse.bass_utils import run_bass_kernel_spmd

NCORES = 8
B = 256            # batch (matmul moving dim)
D = 150528         # 3*224*224
DS = D // NCORES   # 18816 features per core
P = 128            # partitions / contraction tile
J = DS // P        # 147 matmul chunks per core
# j-chunks per DMA. Two competing effects, both measured: (a) entries
# with small descriptors pay a ~1.3us dispatch floor, which binds when
# both queues go small simultaneously — so only mildly small entries at
# the head and a single 2-chunk tail entry; (b) the two HWDGE queues
# share a dispatch arbiter and need MANY comparably-sized entries
# interleaved to stay fair — too few/too big entries (<= ~10 per queue)
# makes one queue hog dispatch and starves the PE.
# v2: small head (4, 12) — NTFF traces show the machine spends ~20% of
# time in a 0.5x engine-throttle phase where the PE (not DMA) sets the
# finish time; a small first entry starts the PE ~3.5us earlier. The
# tail (7, 2) and all-even entry starts are preserved so the DoubleRow
# pairing — and therefore the numerics — is bit-identical to the
# measured-safe baseline. Even/odd chunk totals are 74/73 so the two
# queues carry near-equal bytes (7.54/7.51 MB).
SIZES = [4, 12, 14, 18, 18, 18, 18, 18, 18, 7, 2]
# NOTE two measured dead ends, do not repeat: (1) consuming entries out
# of j-order (to dodge the tail entry's dispatch floor) changes the PSUM
# accumulation order and pushed max rel error 1.962e-2 -> 2.009e-2, OVER
# the 2e-2 gate; (2) reordering only the ISSUE order (numerics-safe,
# d=10's DMAs issued early) regressed 56.8 -> 62.5us — the two queues'
# entry-size interleave alignment matters to the dispatch arbiter.
# Ascending order for both, as below, is the measured optimum.
assert sum(SIZES) == J
STARTS = [sum(SIZES[:i]) for i in range(len(SIZES))]
NDMA = len(SIZES)
W16C = 8 * J       # fp16 weight cols (wh|wl, 4 each, per chunk)
# fp8 weights: 4 cols per chunk, tightly packed. (The old DoubleRow path
# needed a 16 B k-tile pad; plain fp8 matmuls on column tiles don't, which
# also cuts 225 KB/core of stream.)
W8P = 4
W8C = W8P * J      # fp8 weight cols (16 per chunk, 4 used)
T16 = W16C + J * B
T8 = W8C + J * B
XS = 2.0 ** 12     # fp8 residual scale
WS = 2.0 ** 9      # fp8 weight scale
THRESHOLD = 0.95

_CACHE = {}


def _build():
    nc = bass.Bass()
    x16_in = nc.declare_dram_parameter("x16", [P, T16], mybir.dt.float16, isOutput=False)
    x8_in = nc.declare_dram_parameter("x8", [P, T8], mybir.dt.float8e4, isOutput=False)
    # one merged psum image: rows 0:8 = fp16 logits (column tile T0), rows
    # 32:36 / 64:68 / 96:100 = fp8 residual regions (tiles T1/T2/T3)
    out = nc.declare_dram_parameter(
        "partial", [100, B], mybir.dt.float32, isOutput=True
    )

    with ExitStack() as ctx:
        w16 = ctx.enter_context(nc.sbuf_tensor("w16", [P, W16C], mybir.dt.float16))
        # fp8 tiles are 3D ([P, chunks, ...]) so a [P, 2, .] slice of two
        # consecutive chunks feeds a DoubleRow matmul directly — the layout
        # in memory is unchanged
        w8 = ctx.enter_context(nc.sbuf_tensor("w8", [P, J, W8P], mybir.dt.float8e4))
        t16 = []
        t8 = []
        for d in range(NDMA):
            t16.append(
                ctx.enter_context(
                    nc.sbuf_tensor(f"t16_{d}", [P, SIZES[d] * B], mybir.dt.float16)
                )
            )
            t8.append(
                ctx.enter_context(
                    nc.sbuf_tensor(f"t8_{d}", [P, SIZES[d], B], mybir.dt.float8e4)
                )
            )
        out_sb = ctx.enter_context(
            nc.sbuf_tensor("out_sb", [100, B], mybir.dt.float32)
        )
        # One psum tensor, four disjoint column-tile regions. fp16 stays a
        # single chain on tile T0 (partitions 0:8) — its accumulation order
        # is bit-identical to the measured-safe baseline (a 4-way fp16
        # region split was tried: rel err 2.095e-2 > 2e-2 gate). The fp8
        # matmuls round-robin over tiles T1/T2/T3 (partitions 32/64/96) and
        # execute CONCURRENTLY with the fp16 chain — removing ~8us of
        # serial PE time. Splitting the residual accumulation into 3 chains
        # shifts the logits by ~1e-10 (scaled by 2^21, divided on host).
        acc = ctx.enter_context(nc.psum_tensor("acc", [100, B], mybir.dt.float32))

        sw16 = ctx.enter_context(nc.semaphore("sw16"))
        sw8 = ctx.enter_context(nc.semaphore("sw8"))
        s16 = [ctx.enter_context(nc.semaphore(f"s16_{d}")) for d in range(NDMA)]
        s8 = [ctx.enter_context(nc.semaphore(f"s8_{d}")) for d in range(NDMA)]
        pe16_sem = ctx.enter_context(nc.semaphore("pe16"))
        pe8_sem = ctx.enter_context(nc.semaphore("pe8"))
        dve_sem = ctx.enter_context(nc.semaphore("dve"))
        osem = ctx.enter_context(nc.semaphore("o"))

        # per-stream chunk DMAs alternate engines so each HWDGE engine's
        # queue set carries ~equal bytes (a lopsided split runs one stream
        # at half rate and starves the PE). Entry 0's parity is flipped:
        # t16_0 rides the scalar queue behind the tiny 75KB w8 (instead of
        # behind the 301KB w16), so the PE's first matmul starts ~1us
        # earlier; the flip also balances the queues at 7.41/7.42 MB.
        def issue_x(eng, parity):
            for d in range(NDMA):
                do16 = ((d % 2 == 0) != (d == 0)) == (parity == "even")
                if do16:
                    c0 = W16C + STARTS[d] * B
                    eng.dma_start(
                        t16[d][:], x16_in[:, c0 : c0 + SIZES[d] * B]
                    ).then_inc(s16[d], 16)
                else:
                    c0 = W8C + STARTS[d] * B
                    eng.dma_start(
                        t8[d][:], x8_in[:, c0 : c0 + SIZES[d] * B]
                    ).then_inc(s8[d], 16)

        block = ctx.enter_context(nc.Block())

        # No self-init sem clears / pseudo-barrier: the NRT-injected NEFF
        # postamble (observed in every NTFF trace, pc-space below the kernel)
        # zeroes semaphores S[3..255] across all five engines after EVERY
        # NEFF execution, so bass sems are guaranteed 0 at kernel entry.
        # Dropping the defensive init moves the first DMA issue ~1us earlier.
        # (test.py's double-run exercises the re-execution path on HW.)
        nc.sync.dma_start(w16[:], x16_in[:, 0:W16C]).then_inc(sw16, 16)
        issue_x(nc.sync, "even")
        nc.scalar.dma_start(w8[:], x8_in[:, 0:W8C]).then_inc(sw8, 16)
        issue_x(nc.scalar, "odd")

        @block.tensor
        def _(tensor):
            # The PE issues one 256-row matmul per ~108ns; each fp16<->fp8
            # mode switch costs an extra ~100ns (LDWEIGHTS can't prefetch
            # across the switch). Grouping entry PAIRS — fp16(d), fp16(d+1),
            # fp8(d), fp8(d+1) — halves the switch count while keeping BOTH
            # accumulation chains in strictly ascending j order and the
            # DoubleRow pairing untouched, so the numerics are bit-identical.
            # The fp16 matmuls of entry d wait only on s16[d] (not s8[d]),
            # which starts the PE ~1us earlier.
            last_j = J - 1
            tensor.wait_ge(sw16, 16)
            # sw8 wait sits before the FIRST fp8 matmul (not at the top), so
            # the first fp16 matmul is gated only on sw16 + s16_0
            sw8_waited = False
            i8 = 0  # fp8 matmul emission index, for T1/T2/T3 round-robin
            # (0,) alone so the PE's first matmul is gated only on the small
            # head entry; (9, 10) last so pe16 (after fp16(10)) still has
            # fp8(9)+fp8(10) (~5 matmuls) in flight for the DVE to hide
            # the acc16 copy under.
            groups = [(0,)] + [(d, d + 1) for d in range(1, NDMA, 2)]
            for group in groups:
                for d in group:
                    tensor.wait_ge(s16[d], 16)
                    for jj in range(SIZES[d]):
                        j = STARTS[d] + jj
                        mm16 = tensor.matmul(
                            acc[0:8],
                            w16[:, 8 * j : 8 * j + 8],
                            t16[d][:, jj * B : jj * B + B],
                            start=(j == 0),
                            stop=(j == last_j),
                        )
                # fp8 residual stream: plain fp8 matmuls (DoubleRow is only
                # valid at column quadrant 0 — ISA s3d3_mm_valid_dst_partition)
                # round-robin over column tiles T1/T2/T3, which execute
                # concurrently with the fp16 chain on T0.
                for d in group:
                    if not sw8_waited:
                        tensor.wait_ge(sw8, 16)
                        sw8_waited = True
                    tensor.wait_ge(s8[d], 16)
                    for jj in range(SIZES[d]):
                        j = STARTS[d] + jj
                        r = 32 * (1 + i8 % 3)
                        mm = tensor.matmul(
                            acc[r : r + 4],
                            w8[:, j : j + 1, 0:4],
                            t8[d][:, jj : jj + 1, :],
                            start=(i8 < 3),
                            stop=(i8 >= J - 3),
                            tile_position=(0, r),
                        )
                        i8 += 1
            # last fp16 matmul (j=146, entry 10) retires BEFORE the final fp8
            # group — pe16 lets the DVE copy acc16 under the last ~5 fp8
            # matmuls; pe8 fires at the true end.
            mm16.then_inc(pe16_sem, 1)
            mm.then_inc(pe8_sem, 1)

        @block.vector
        def _(vector):
            # DVE (not the Activation engine): tensor_copy needs no
            # ACT_TABLE_LOAD, and the cross-engine dve_sem doubles as the
            # ordering fence for the out DMA — engines run relaxed, and a
            # same-engine dma_start executes AHEAD of in-flight copies and
            # ships stale out_sb (seen on hardware).
            vector.wait_ge(pe16_sem, 1)
            vector.wait_ge(pe8_sem, 1)
            # small DVE-side delay: the matmul's sem update fires at retire,
            # ~128 PE cycles before the systolic drain lands in PSUM. The
            # dummy cast lands in a cell the acc copy overwrites next.
            vector.tensor_copy(out_sb[0:1, 0:8], w16[0:1, 0:8])
            # one copy for all four psum regions — DVE lanes run
            # per-partition, so 100 partitions cost the same as 8
            vector.tensor_copy(out_sb[:], acc[:]).then_inc(dve_sem, 1)

        @block.sync
        def _(sync):
            # out DMA on sync (idle since its issues): its DMA_SEQ is
            # cheaper than scalar's, and the exec-ending pseudo-barrier is
            # gated by whichever engine issues this — no wait on osem: the
            # Block-exit DRAIN on this engine quiesces its HWDGE queue,
            # which covers the out DMA's completion
            sync.wait_ge(dve_sem, 1)
            sync.dma_start(out[:], out_sb[:]).then_inc(osem, 16)

    return nc


def _pack(x, W1, W2):
    xf = np.ascontiguousarray(x, dtype=np.float32).reshape(B, D)
    xh = xf.astype(np.float16)
    xr8 = ((xf - xh.astype(np.float32)) * np.float32(XS)).astype(ml_dtypes.float8_e4m3)

    w4 = np.concatenate(
        [np.asarray(W1, np.float32), np.asarray(W2, np.float32)], axis=1
    )  # [D, 4]
    wh = w4.astype(np.float16)
    wl = (w4 - wh.astype(np.float32)).astype(np.float16)
    w8 = (w4 * np.float32(WS)).astype(ml_dtypes.float8_e4m3)

    xw16 = np.empty((NCORES, P, T16), dtype=np.float16)
    # fp16 W part: col 8j + h*4 + c = (wh,wl)[h][k*DS + j*P + p, c]
    wst = np.stack([wh, wl])  # [2, D, 4]
    xw16[:, :, :W16C] = (
        wst.reshape(2, NCORES, J, P, 4)
        .transpose(1, 3, 2, 0, 4)
        .reshape(NCORES, P, W16C)
    )
    # fp16 x part: col W16C + j*B + b = xh[b, k*DS + j*P + p]
    xw16[:, :, W16C:] = (
        xh.reshape(B, NCORES, J, P).transpose(1, 3, 2, 0).reshape(NCORES, P, J * B)
    )

    xw8 = np.empty((NCORES, P, T8), dtype=ml_dtypes.float8_e4m3)
    xw8[:, :, :W8C] = (
        w8.reshape(NCORES, J, P, 4).transpose(0, 2, 1, 3).reshape(NCORES, P, W8C)
    )
    xw8[:, :, W8C:] = (
        xr8.reshape(B, NCORES, J, P).transpose(1, 3, 2, 0).reshape(NCORES, P, J * B)
    )
    return xw16, xw8


def kernel(x, W1, b1, W2, b2, trace=False, trace_cores=None):
    if "nc" not in _CACHE:
        _CACHE["nc"] = _build()
    nc = _CACHE["nc"]

    xw16, xw8 = _pack(x, W1, W2)
    in_maps = [{"x16": xw16[k], "x8": xw8[k]} for k in range(NCORES)]
    kw = {"trace_cores": trace_cores} if trace_cores else {}
    res = run_bass_kernel_spmd(nc, in_maps, list(range(NCORES)), trace=trace, **kw)
    _CACHE["last_results"] = res

    logits4 = np.zeros((4, B), dtype=np.float64)
    for k in range(NCORES):
        r = res.results[k]["partial"]  # [100, 256]
        logits4 += r[0:4] + r[4:8]
        for reg in (32, 64, 96):
            logits4 += r[reg : reg + 4].astype(np.float64) / (XS * WS)
    logits4 = logits4.astype(np.float32)

    t_out = logits4[0:2].T + np.asarray(b1, np.float32)  # [256, 2]
    f_out = logits4[2:4].T + np.asarray(b2, np.float32)  # [256, 2]
    m = t_out.max(axis=1, keepdims=True)
    e = np.exp(t_out - m)
    conf = (e / e.sum(axis=1, keepdims=True)).max(axis=1)
    blended = 0.7 * t_out + 0.3 * f_out
    out = np.where((conf > THRESHOLD)[:, None], t_out, blended)
    return out.astype(np.float32)



# revision 38
# speedup vs baseline: 1.0970x; 1.0970x over previous
"""Trainium2 Bass kernel for nn_CascadingSystem (confidence-gated 2-expert blend).

Computation (reference):
    xf = x.reshape(256, 150528)
    t_out = xf @ W1 + b1            # [256, 2]
    f_out = xf @ W2 + b2            # [256, 2]
    conf  = max(softmax(t_out, 1), 1)
    out   = where(conf > 0.95, t_out, 0.7*t_out + 0.3*f_out)

Strategy (memory-bound; reading x dominates; ~400-425 B/ns DMA/core):
  - Shard the feature dim D=150528 across 8 cores (18816 each). Every core
    streams its d-slice of ALL 256 samples once from HBM and computes the
    partial [4, 256] logits (4 = W1c0, W1c1, W2c0, W2c1) on the tensor
    engine, 147 accumulating matmul chunks of K=128.
  - Precision/bandwidth: fp32 matmuls are 4 cyc/row (PE-bound) and fp32
    data is 4 B/elem. Instead decompose on the host
        x = xh(fp16) + xr,   xr8 = fp8_e4m3(xr * 2^12)
        W = wh(fp16) + wl(fp16),  w8 = fp8_e4m3(W * 2^9)
        logits = xh*wh + xh*wl + (xr8*w8) / 2^21
    3 B/elem -> ~36us stream. Max logit error ~1.2e-4, final max rel err
    1.9642e-2 vs the 2e-2 gate. The binding element is out[120,0] with
    only ~2% error headroom: the fp16 chain's PSUM accumulation order
    must stay exactly ascending-j in ONE chain (a 4-way region split of
    the fp16 accumulation measured 2.095e-2 — over the gate).
  - PE column tiling: every matmul is 8 or 4 stationary cols of a 128-col
    array, so the array runs as four 128x32 column tiles. The fp16 chain
    owns tile T0 (psum partitions 0:8, order-preserving); the 147 plain
    fp8 residual matmuls round-robin tiles T1/T2/T3 (psum partitions
    32/64/96, tile_position explicit) and execute CONCURRENTLY with the
    fp16 chain — PE time drops from ~25.5us serial to ~16-17us, below
    the stream pace even in the machine's 0.5x-throttle phases.
    (DoubleRow fp8 was used before: it is ISA-restricted to column
    quadrant 0 — s3d3_mm_valid_dst_partition — so it cannot tile; plain
    fp8 also drops DR's 16 B k-tile weight pad, saving 225 KB of stream.)
    Splitting the residual accumulation into 3 chains shifts the logits
    by ~1e-10 (the chain is scaled by 2^21 and divided back on host).
  - DMA structure (measured on HW): the two HWDGE queues (sync + scalar)
    share a descriptor-dispatch arbiter; sustained ~415 B/ns needs BOTH
    queues active with many comparable, mid-size entries interleaved —
    see SIZES for the head/tail tapering rationale. Per-stream chunk DMAs
    alternate engines per entry; W goes first on each queue.
  - No self-init sem clears / NRT pseudo-barrier: the runtime-injected
    NEFF postamble (observed in every NTFF trace at low PCs) zeroes
    semaphores S[3..255] after EVERY execution, so bass sems are
    guaranteed 0 at kernel entry. Dropping the defensive init moves the
    first DMA issue ~1us earlier.
  - Epilogue: one merged PSUM image [100, 256] (fp16 rows 0:8 + three fp8
    regions) -> one DVE copy -> one out DMA. DVE (not Activation): no
    ACT_TABLE_LOAD, and its dve_sem doubles as the ordering fence for
    sync's out DMA — engines run relaxed, and a same-engine dma_start
    executes AHEAD of in-flight copies and ships stale out_sb (seen on
    HW). One dummy DVE cast covers the ~50ns between matmul retire (sem
    update) and the systolic drain landing in PSUM. Nobody waits on the
    out DMA's sem: the Block-exit DRAIN on sync quiesces its HWDGE
    dispatch, and the postamble runs long past the transfer.
  - Host sums the 8 cores x 4 regions in float64 and applies the tiny
    bias/softmax/threshold/blend epilogue on [256, 4] floats.

Measured (2026-08-10): 52.4-53.9us normal phase, ~58.5us in throttle
phases (machine drifts; ~15-20% of time at a 0.5x engine-util limit).
Fixed overheads inside the measured window: ~7.5us runtime postamble
(per-engine serial clear of ~51 sems each + exit barriers, injected at
NEFF load — not removable from the kernel) and ~1.5us HWDGE first-byte
latency at the head. Baseline this session started from: 59.3-61.2us.
"""

from contextlib import ExitStack

import ml_dtypes
import numpy as np

import concourse.bass as bass
import concourse.mybir as mybir
from concourse.bass_utils import run_bass_kernel_spmd

NCORES = 8
B = 256            # batch (matmul moving dim)
D = 150528         # 3*224*224
DS = D // NCORES   # 18816 features per core
P = 128            # partitions / contraction tile
J = DS // P        # 147 matmul chunks per core
# j-chunks per DMA. Two competing effects, both measured: (a) entries
# with small descriptors pay a ~1.3us dispatch floor, which binds when
# both queues go small simultaneously — so only mildly small entries at
# the head and a single 2-chunk tail entry; (b) the two HWDGE queues
# share a dispatch arbiter and need MANY comparably-sized entries
# interleaved to stay fair — too few/too big entries (<= ~10 per queue)
# makes one queue hog dispatch and starves the PE.
# Small head (4, 12): NTFF traces show the machine spends ~15-20% of
# time in a 0.5x engine-throttle phase where the PE (not DMA) sets the
# finish time; a small first entry starts the PE ~3.5us earlier. Tapered
# tail (10, 7, 4, 2): the PE runs ~2-4us behind the stream in throttle
# phases, so finer tail entries hand it the last chunks sooner. (Entry
# boundaries don't affect matmul order — numerics unchanged.)
SIZES = [4, 12, 18, 18, 18, 18, 18, 18, 10, 7, 4, 2]
# NOTE two measured dead ends, do not repeat: (1) consuming entries out
# of j-order (to dodge the tail entry's dispatch floor) changes the PSUM
# accumulation order and pushed max rel error 1.962e-2 -> 2.009e-2, OVER
# the 2e-2 gate; (2) reordering only the ISSUE order (numerics-safe,
# d=10's DMAs issued early) regressed 56.8 -> 62.5us — the two queues'
# entry-size interleave alignment matters to the dispatch arbiter.
# Ascending order for both, as below, is the measured optimum.
assert sum(SIZES) == J
STARTS = [sum(SIZES[:i]) for i in range(len(SIZES))]
NDMA = len(SIZES)
W16C = 8 * J       # fp16 weight cols (wh|wl, 4 each, per chunk)
# fp8 weights: 4 cols per chunk, tightly packed. (The old DoubleRow path
# needed a 16 B k-tile pad; plain fp8 matmuls on column tiles don't, which
# also cuts 225 KB/core of stream.)
W8P = 4
W8C = W8P * J      # fp8 weight cols (16 per chunk, 4 used)
T16 = W16C + J * B
T8 = W8C + J * B
XS = 2.0 ** 12     # fp8 residual scale
WS = 2.0 ** 9      # fp8 weight scale
THRESHOLD = 0.95

_CACHE = {}


def _build():
    nc = bass.Bass()
    x16_in = nc.declare_dram_parameter("x16", [P, T16], mybir.dt.float16, isOutput=False)
    x8_in = nc.declare_dram_parameter("x8", [P, T8], mybir.dt.float8e4, isOutput=False)
    # one merged psum image: rows 0:8 = fp16 logits (column tile T0), rows
    # 32:36 / 64:68 / 96:100 = fp8 residual regions (tiles T1/T2/T3)
    out = nc.declare_dram_parameter(
        "partial", [100, B], mybir.dt.float32, isOutput=True
    )

    with ExitStack() as ctx:
        w16 = ctx.enter_context(nc.sbuf_tensor("w16", [P, W16C], mybir.dt.float16))
        # fp8 tiles are 3D ([P, chunks, ...]) so a [P, 2, .] slice of two
        # consecutive chunks feeds a DoubleRow matmul directly — the layout
        # in memory is unchanged
        w8 = ctx.enter_context(nc.sbuf_tensor("w8", [P, J, W8P], mybir.dt.float8e4))
        t16 = []
        t8 = []
        for d in range(NDMA):
            t16.append(
                ctx.enter_context(
                    nc.sbuf_tensor(f"t16_{d}", [P, SIZES[d] * B], mybir.dt.float16)
                )
            )
            t8.append(
                ctx.enter_context(
                    nc.sbuf_tensor(f"t8_{d}", [P, SIZES[d], B], mybir.dt.float8e4)
                )
            )
        out_sb = ctx.enter_context(
            nc.sbuf_tensor("out_sb", [100, B], mybir.dt.float32)
        )
        # One psum tensor, four disjoint column-tile regions. fp16 stays a
        # single chain on tile T0 (partitions 0:8) — its accumulation order
        # is bit-identical to the measured-safe baseline (a 4-way fp16
        # region split was tried: rel err 2.095e-2 > 2e-2 gate). The fp8
        # matmuls round-robin over tiles T1/T2/T3 (partitions 32/64/96) and
        # execute CONCURRENTLY with the fp16 chain — removing ~8us of
        # serial PE time. Splitting the residual accumulation into 3 chains
        # shifts the logits by ~1e-10 (scaled by 2^21, divided on host).
        acc = ctx.enter_context(nc.psum_tensor("acc", [100, B], mybir.dt.float32))

        sw16 = ctx.enter_context(nc.semaphore("sw16"))
        sw8 = ctx.enter_context(nc.semaphore("sw8"))
        s16 = [ctx.enter_context(nc.semaphore(f"s16_{d}")) for d in range(NDMA)]
        s8 = [ctx.enter_context(nc.semaphore(f"s8_{d}")) for d in range(NDMA)]
        pe16_sem = ctx.enter_context(nc.semaphore("pe16"))
        pe8_sem = ctx.enter_context(nc.semaphore("pe8"))
        dve_sem = ctx.enter_context(nc.semaphore("dve"))
        osem = ctx.enter_context(nc.semaphore("o"))

        # per-stream chunk DMAs alternate engines so each HWDGE engine's
        # queue set carries ~equal bytes (a lopsided split runs one stream
        # at half rate and starves the PE). Entry 0's parity is flipped:
        # t16_0 rides the scalar queue behind the tiny 75KB w8 (instead of
        # behind the 301KB w16), so the PE's first matmul starts ~1us
        # earlier; flipping the last entry too balances the queues at
        # 7.42/7.42 MB.
        def issue_x(eng, parity):
            for d in range(NDMA):
                do16 = ((d % 2 == 0) != (d in (0, NDMA - 1))) == (parity == "even")
                if do16:
                    c0 = W16C + STARTS[d] * B
                    eng.dma_start(
                        t16[d][:], x16_in[:, c0 : c0 + SIZES[d] * B]
                    ).then_inc(s16[d], 16)
                else:
                    c0 = W8C + STARTS[d] * B
                    eng.dma_start(
                        t8[d][:], x8_in[:, c0 : c0 + SIZES[d] * B]
                    ).then_inc(s8[d], 16)

        block = ctx.enter_context(nc.Block())

        # No self-init sem clears / pseudo-barrier: the NRT-injected NEFF
        # postamble (observed in every NTFF trace, pc-space below the kernel)
        # zeroes semaphores S[3..255] across all five engines after EVERY
        # NEFF execution, so bass sems are guaranteed 0 at kernel entry.
        # Dropping the defensive init moves the first DMA issue ~1us earlier.
        # (test.py's double-run exercises the re-execution path on HW.)
        nc.sync.dma_start(w16[:], x16_in[:, 0:W16C]).then_inc(sw16, 16)
        issue_x(nc.sync, "even")
        nc.scalar.dma_start(w8[:], x8_in[:, 0:W8C]).then_inc(sw8, 16)
        issue_x(nc.scalar, "odd")

        @block.tensor
        def _(tensor):
            # The PE issues one 256-row matmul per ~108ns; each fp16<->fp8
            # mode switch costs an extra ~100ns (LDWEIGHTS can't prefetch
            # across the switch). Grouping entry PAIRS — fp16(d), fp16(d+1),
            # fp8(d), fp8(d+1) — halves the switch count while keeping BOTH
            # accumulation chains in strictly ascending j order and the
            # DoubleRow pairing untouched, so the numerics are bit-identical.
            # The fp16 matmuls of entry d wait only on s16[d] (not s8[d]),
            # which starts the PE ~1us earlier.
            last_j = J - 1
            tensor.wait_ge(sw16, 16)
            # sw8 wait sits before the FIRST fp8 matmul (not at the top), so
            # the first fp16 matmul is gated only on sw16 + s16_0
            sw8_waited = False
            i8 = 0  # fp8 matmul emission index, for T1/T2/T3 round-robin
            # (0,) alone so the PE's first matmul is gated only on the small
            # head entry; entries then pair up to halve fp16<->fp8 mode
            # transitions (each switch costs ~100ns of PE issue time).
            groups = [(0,)] + [
                tuple(g for g in (d, d + 1) if g < NDMA) for d in range(1, NDMA, 2)
            ]
            for group in groups:
                for d in group:
                    tensor.wait_ge(s16[d], 16)
                    for jj in range(SIZES[d]):
                        j = STARTS[d] + jj
                        mm16 = tensor.matmul(
                            acc[0:8],
                            w16[:, 8 * j : 8 * j + 8],
                            t16[d][:, jj * B : jj * B + B],
                            start=(j == 0),
                            stop=(j == last_j),
                        )
                # fp8 residual stream: plain fp8 matmuls (DoubleRow is only
                # valid at column quadrant 0 — ISA s3d3_mm_valid_dst_partition)
                # round-robin over column tiles T1/T2/T3, which execute
                # concurrently with the fp16 chain on T0.
                for d in group:
                    if not sw8_waited:
                        tensor.wait_ge(sw8, 16)
                        sw8_waited = True
                    tensor.wait_ge(s8[d], 16)
                    for jj in range(SIZES[d]):
                        j = STARTS[d] + jj
                        r = 32 * (1 + i8 % 3)
                        mm = tensor.matmul(
                            acc[r : r + 4],
                            w8[:, j : j + 1, 0:4],
                            t8[d][:, jj : jj + 1, :],
                            start=(i8 < 3),
                            stop=(i8 >= J - 3),
                            tile_position=(0, r),
                        )
                        i8 += 1
            # last fp16 matmul (j=146, entry 10) retires BEFORE the final fp8
            # group — pe16 lets the DVE copy acc16 under the last ~5 fp8
            # matmuls; pe8 fires at the true end.
            mm16.then_inc(pe16_sem, 1)
            mm.then_inc(pe8_sem, 1)

        @block.vector
        def _(vector):
            # DVE (not the Activation engine): tensor_copy needs no
            # ACT_TABLE_LOAD, and the cross-engine dve_sem doubles as the
            # ordering fence for the out DMA — engines run relaxed, and a
            # same-engine dma_start executes AHEAD of in-flight copies and
            # ships stale out_sb (seen on hardware).
            vector.wait_ge(pe16_sem, 1)
            vector.wait_ge(pe8_sem, 1)
            # small DVE-side delay: the matmul's sem update fires at retire,
            # ~128 PE cycles before the systolic drain lands in PSUM. The
            # dummy cast lands in a cell the acc copy overwrites next.
            vector.tensor_copy(out_sb[0:1, 0:8], w16[0:1, 0:8])
            # one copy for all four psum regions — DVE lanes run
            # per-partition, so 100 partitions cost the same as 8
            vector.tensor_copy(out_sb[:], acc[:]).then_inc(dve_sem, 1)

        @block.sync
        def _(sync):
            # out DMA on sync (idle since its issues): its DMA_SEQ is
            # cheaper than scalar's, and the exec-ending pseudo-barrier is
            # gated by whichever engine issues this — no wait on osem: the
            # Block-exit DRAIN on this engine quiesces its HWDGE queue,
            # which covers the out DMA's completion
            sync.wait_ge(dve_sem, 1)
            sync.dma_start(out[:], out_sb[:]).then_inc(osem, 16)

    return nc


def _pack(x, W1, W2):
    xf = np.ascontiguousarray(x, dtype=np.float32).reshape(B, D)
    xh = xf.astype(np.float16)
    xr8 = ((xf - xh.astype(np.float32)) * np.float32(XS)).astype(ml_dtypes.float8_e4m3)

    w4 = np.concatenate(
        [np.asarray(W1, np.float32), np.asarray(W2, np.float32)], axis=1
    )  # [D, 4]
    wh = w4.astype(np.float16)
    wl = (w4 - wh.astype(np.float32)).astype(np.float16)
    w8 = (w4 * np.float32(WS)).astype(ml_dtypes.float8_e4m3)

    xw16 = np.empty((NCORES, P, T16), dtype=np.float16)
    # fp16 W part: col 8j + h*4 + c = (wh,wl)[h][k*DS + j*P + p, c]
    wst = np.stack([wh, wl])  # [2, D, 4]
    xw16[:, :, :W16C] = (
        wst.reshape(2, NCORES, J, P, 4)
        .transpose(1, 3, 2, 0, 4)
        .reshape(NCORES, P, W16C)
    )
    # fp16 x part: col W16C + j*B + b = xh[b, k*DS + j*P + p]
    xw16[:, :, W16C:] = (
        xh.reshape(B, NCORES, J, P).transpose(1, 3, 2, 0).reshape(NCORES, P, J * B)
    )

    xw8 = np.empty((NCORES, P, T8), dtype=ml_dtypes.float8_e4m3)
    xw8[:, :, :W8C] = (
        w8.reshape(NCORES, J, P, 4).transpose(0, 2, 1, 3).reshape(NCORES, P, W8C)
    )
    xw8[:, :, W8C:] = (
        xr8.reshape(B, NCORES, J, P).transpose(1, 3, 2, 0).reshape(NCORES, P, J * B)
    )
    return xw16, xw8


def kernel(x, W1, b1, W2, b2, trace=False, trace_cores=None):
    if "nc" not in _CACHE:
        _CACHE["nc"] = _build()
    nc = _CACHE["nc"]

    xw16, xw8 = _pack(x, W1, W2)
    in_maps = [{"x16": xw16[k], "x8": xw8[k]} for k in range(NCORES)]
    kw = {"trace_cores": trace_cores} if trace_cores else {}
    res = run_bass_kernel_spmd(nc, in_maps, list(range(NCORES)), trace=trace, **kw)
    _CACHE["last_results"] = res

    logits4 = np.zeros((4, B), dtype=np.float64)
    for k in range(NCORES):
        r = res.results[k]["partial"]  # [100, 256]
        logits4 += r[0:4] + r[4:8]
        for reg in (32, 64, 96):
            logits4 += r[reg : reg + 4].astype(np.float64) / (XS * WS)
    logits4 = logits4.astype(np.float32)

    t_out = logits4[0:2].T + np.asarray(b1, np.float32)  # [256, 2]
    f_out = logits4[2:4].T + np.asarray(b2, np.float32)  # [256, 2]
    m = t_out.max(axis=1, keepdims=True)
    e = np.exp(t_out - m)
    conf = (e / e.sum(axis=1, keepdims=True)).max(axis=1)
    blended = 0.7 * t_out + 0.3 * f_out
    out = np.where((conf > THRESHOLD)[:, None], t_out, blended)
    return out.astype(np.float32)

